# revision 11
# baseline (speedup 1.0000x reference)
"""Binarized complex-style dense layer on 8 TRN2 NeuronCores.

Computes out = sign(x + eps) @ K^T with K = [[br, -bi], [bi, br]],
br = sign(weight_real + eps), bi = sign(weight_imag + eps).

Sharding: data-parallel over the batch dim (131072 rows -> 16384 per core),
weights replicated.  Forward only, so no collectives.

Each core receives its x shard TRANSPOSED (k-major, [256, 16384] bf16,
host-side +eps fold keeps the bf16 cast sign-exact) and produces the
transposed output (out/2 as int8, [256, 16384]); the host undoes both.
With x in k-major the matmul runs weights-stationary:

  DMA in   : per 2048-col chunk one 1 MB load, 2 x 4 KB runs/partition
  binarize : one DVE tensor_scalar per chunk: (x >= 0) - 0.5 -> {-.5,+.5}
             (kernel weights scaled to {-2,+2} so products are exactly +-1)
  PE       : rhs = binarized x streams N=512 columns; stationary cycles
             through the four 128x128 pieces of kernelT (2 k-halves x
             2 o-halves), 4 LDW + 16 MM per chunk
  PSUM     : outT [o, b] f32, exact even ints in [-256, 256]
  copy     : ACT/DVE f32 -> int8 with scale 0.5 over [128, 1024] 2-bank APs
  DMA out  : per-chunk store, 2 x 2 KB runs per partition
"""

import sys

import numpy as np

try:
    import concourse.bass  # noqa: F401
except ImportError:  # fresh env without the axon PYTHONPATH entries
    for p in ("/root/.axon_site/_ro/trn_rl_repo", "/opt/trn_rl_repo"):
        if p not in sys.path:
            sys.path.append(p)

import ml_dtypes

N_CORES = 8
B_TOTAL = 131072
ROWS_PER_CORE = B_TOTAL // N_CORES  # 16384
FAN = 128
K2 = 2 * FAN  # 256 = 2*fan_in = 2*fan_out
EPS = 1e-6
CHUNKS = [1024] + [2048] * 7 + [1024]
assert sum(CHUNKS) == ROWS_PER_CORE

_NC_CACHE = {}


def _build_nc(rows_per_core):
    from concourse import bacc, mybir, tile

    f32 = mybir.dt.float32
    bf16 = mybir.dt.bfloat16
    i8 = mybir.dt.int8
    Sign = mybir.ActivationFunctionType.Sign
    Copy = mybir.ActivationFunctionType.Copy
    Alu = mybir.AluOpType

    assert rows_per_core == ROWS_PER_CORE
    starts = [sum(CHUNKS[:i]) for i in range(len(CHUNKS))]

    nc = bacc.Bacc("TRN2", target_bir_lowering=False, debug=False)

    # x arrives k-major: row k (0..255), column b = batch row within shard.
    x_d = nc.dram_tensor("x", [K2, rows_per_core], bf16, kind="ExternalInput")
    # weights arrive pre-transposed: wrt[k, o] = weight_real[o, k].
    wrt_d = nc.dram_tensor("weight_real_t", [FAN, FAN], f32, kind="ExternalInput")
    wit_d = nc.dram_tensor("weight_imag_t", [FAN, FAN], f32, kind="ExternalInput")
    # out is produced transposed: out_d[o, b] = out[b, o] / 2.
    out_d = nc.dram_tensor("out", [K2, rows_per_core], i8, kind="ExternalOutput")

    with tile.TileContext(nc) as tc:
        with (
            tc.tile_pool(name="const", bufs=1) as const_pool,
            tc.tile_pool(name="xt", bufs=len(CHUNKS)) as xt_pool,
            tc.tile_pool(name="xb", bufs=3) as xb_pool,
            tc.tile_pool(name="oout", bufs=3) as o_pool,
            tc.tile_pool(name="po", bufs=4, space="PSUM") as po_pool,
        ):
            # Per-chunk input tiles [128, (h, w)]: k-half h on cols h*w + b.
            xt_tiles = []

            def load(c):
                s, w = starts[c], CHUNKS[c]
                xt = xt_pool.tile([128, 2 * w], bf16, tag="xt")
                nc.sync.dma_start(
                    out=xt[:].rearrange("p (h b) -> p h b", h=2),
                    in_=x_d.rearrange("(h p) b -> p h b", h=2)[:, :, s : s + w],
                )
                xt_tiles.append(xt)

            # Get the x stream going before anything else.
            for c in range(len(CHUNKS)):
                load(c)

            # PE warm-up: ~5us of junk matmuls so the HAM clock gate opens
            # (1.2 -> 2.4 GHz) before the first real matmul arrives.
            warm = const_pool.tile([128, 128], bf16)
            nc.vector.memset(warm[:], 1.0)
            warm_ps = po_pool.tile([128, 1024], f32, tag="po", name="warm_ps")
            for _ in range(40):
                nc.tensor.matmul(
                    warm_ps[:, 0:64], warm[:], warm[:, 0:64], start=True, stop=True
                )

            # kernelT [256 k, 256 o] as two [128, 256] bf16 tiles scaled x2
            # (x is binarized to {-.5,+.5}, so products are exactly +-1):
            #   kt0 = 2*[ sign(wr^T) | sign(wi^T) ]   (k in [0,128))
            #   kt1 = 2*[ -sign(wi^T) | sign(wr^T) ]  (k in [128,256))
            # sign(w + eps) = (w >= -eps) * 2 - 1, computed as two DVE
            # tensor_scalar passes (no ACT table load on the critical path).
            w_sb = const_pool.tile([128, 256], f32)
            nc.scalar.dma_start(out=w_sb[:, 0:128], in_=wrt_d[:])
            nc.scalar.dma_start(out=w_sb[:, 128:256], in_=wit_d[:])
            kt0_t = const_pool.tile([128, 256], bf16)
            kt1_t = const_pool.tile([128, 256], bf16)
            # kt0 halves: +sign;  kt1: [-sign(wiT) | +sign(wrT)]
            nc.vector.tensor_scalar(kt0_t[:], w_sb[:], -EPS, 4.0, Alu.is_ge, Alu.mult)
            nc.vector.tensor_scalar(
                kt1_t[:, 0:128], w_sb[:, 128:256], -EPS, -4.0, Alu.is_ge, Alu.mult
            )
            nc.vector.tensor_scalar(
                kt1_t[:, 128:256], w_sb[:, 0:128], -EPS, 4.0, Alu.is_ge, Alu.mult
            )
            kt0 = const_pool.tile([128, 256], bf16)
            kt1 = const_pool.tile([128, 256], bf16)
            nc.vector.tensor_scalar(kt0[:], kt0_t[:], -2.0, None, Alu.add)
            nc.vector.tensor_scalar(kt1[:, 0:128], kt1_t[:, 0:128], 2.0, None, Alu.add)
            nc.vector.tensor_scalar(kt1[:, 128:256], kt1_t[:, 128:256], -2.0, None, Alu.add)
            kts = (kt0, kt1)

            for c in range(len(CHUNKS)):
                s, w = starts[c], CHUNKS[c]
                xbt = xb_pool.tile([128, 2 * w], bf16, tag="xb")
                # One-shot binarize: (x >= 0) - 0.5 -> {-0.5, +0.5} bf16.
                nc.vector.tensor_scalar(
                    xbt[:], xt_tiles[c][:], 0.0, 0.5, Alu.is_ge, Alu.subtract
                )
                ot = o_pool.tile([128, 2 * w], i8, tag="ot")
                otv = ot[:].rearrange("p (s b) -> p s b", s=2)
                n2 = w // 1024  # 2-bank po tiles per o-half
                for os_half in range(2):
                    pos = [
                        po_pool.tile([128, 1024], f32, tag="po", name=f"po_{c}_{os_half}_{j}")
                        for j in range(n2)
                    ]
                    for h in range(2):
                        # One stationary (128x128 piece of kernelT) streams
                        # all of this chunk's columns: LDW amortizes over
                        # n2*2 N=512 matmuls.
                        lhsT = kts[h][:, os_half * 128 : (os_half + 1) * 128]
                        for j in range(n2):
                            for q in range(2):
                                nc.tensor.matmul(
                                    pos[j][:, q * 512 : (q + 1) * 512],
                                    lhsT,
                                    xbt[:, h * w + j * 1024 + q * 512 : h * w + j * 1024 + (q + 1) * 512],
                                    start=(h == 0),
                                    stop=(h == 1),
                                )
                    for j in range(n2):
                        dst = otv[:, os_half, j * 1024 : (j + 1) * 1024]
                        # ~1.5 copies per chunk on DVE (emitted before the
                        # next chunk's binarize in the FIFO), rest on ACT.
                        on_dve = (os_half == 0 and j == 0) or (
                            os_half == 1 and j == 0 and c % 2 == 1
                        )
                        if on_dve:
                            nc.vector.tensor_scalar(dst, pos[j][:], 0.5, None, Alu.mult)
                        else:
                            nc.scalar.activation(dst, pos[j][:], Copy, bias=0.0, scale=0.5)
                nc.gpsimd.dma_start(
                    out=out_d.rearrange("(s p) b -> p s b", s=2)[:, :, s : s + w],
                    in_=otv,
                )

    nc.compile()
    return nc


def get_nc(rows_per_core=ROWS_PER_CORE):
    if rows_per_core not in _NC_CACHE:
        _NC_CACHE[rows_per_core] = _build_nc(rows_per_core)
    return _NC_CACHE[rows_per_core]


def kernel(x, weight_real, weight_imag, trace=False, tmpdir=None):
    from concourse import bass_utils

    x = np.asarray(x, dtype=np.float32)
    wr = np.asarray(weight_real, dtype=np.float32)
    wi = np.asarray(weight_imag, dtype=np.float32)
    assert x.shape == (B_TOTAL, K2) and wr.shape == (FAN, FAN) and wi.shape == (FAN, FAN)

    # Fold the +eps into the bf16 cast: sign(bf16(x + eps)) == sign(x + eps)
    # (round-to-nearest never crosses 0; exact-0 results go +1 via the
    # device-side >= 0 test, matching sign(0 + eps)).  Feed each core its
    # shard k-major ([256, 16384]); weights go in pre-transposed.
    x_bf = (x + np.float32(EPS)).astype(ml_dtypes.bfloat16)
    xp = np.ascontiguousarray(
        x_bf.reshape(N_CORES, ROWS_PER_CORE, K2).transpose(0, 2, 1)
    )
    wrt = np.ascontiguousarray(wr.T)
    wit = np.ascontiguousarray(wi.T)

    nc = get_nc()
    in_maps = [
        {"x": xp[i], "weight_real_t": wrt, "weight_imag_t": wit}
        for i in range(N_CORES)
    ]
    res = bass_utils.run_bass_kernel_spmd(
        nc, in_maps, core_ids=list(range(N_CORES)), trace=trace, tmpdir=tmpdir
    )
    # out_d[o, b] = out[b, o]/2 per core: untranspose and upcast.
    out = np.empty((B_TOTAL, K2), dtype=np.float32)
    for i in range(N_CORES):
        np.multiply(
            res.results[i]["out"].T, np.float32(2.0),
            out=out[i * ROWS_PER_CORE : (i + 1) * ROWS_PER_CORE],
        )
    if trace:
        return out, res
    return out


# revision 14
# speedup vs baseline: 1.0826x; 1.0826x over previous
"""Binarized complex-style dense layer on 8 TRN2 NeuronCores.

Computes out = sign(x + eps) @ K^T with K = [[br, -bi], [bi, br]],
br = sign(weight_real + eps), bi = sign(weight_imag + eps).

Sharding: data-parallel over the batch dim (131072 rows -> 16384 per core),
weights replicated.  Forward only, so no collectives.

Each core receives its x shard TRANSPOSED (k-major, [256, 16384] bf16,
host-side +eps fold keeps the bf16 cast sign-exact) and produces the
transposed output (out/2 as int8, [256, 16384]); the host undoes both.
With x in k-major the matmul runs weights-stationary:

  DMA in   : per 2048-col chunk one 1 MB load, 2 x 4 KB runs/partition
  binarize : one DVE tensor_scalar per chunk: (x >= 0) - 0.5 -> {-.5,+.5}
             (kernel weights scaled to {-2,+2} so products are exactly +-1)
  PE       : rhs = binarized x streams N=512 columns; stationary cycles
             through the four 128x128 pieces of kernelT (2 k-halves x
             2 o-halves), 4 LDW + 16 MM per chunk
  PSUM     : outT [o, b] f32, exact even ints in [-256, 256]
  copy     : ACT/DVE f32 -> int8 with scale 0.5 over [128, 1024] 2-bank APs
  DMA out  : per-chunk store, 2 x 2 KB runs per partition
"""

import sys

import numpy as np

try:
    import concourse.bass  # noqa: F401
except ImportError:  # fresh env without the axon PYTHONPATH entries
    for p in ("/root/.axon_site/_ro/trn_rl_repo", "/opt/trn_rl_repo"):
        if p not in sys.path:
            sys.path.append(p)

import ml_dtypes

N_CORES = 8
B_TOTAL = 131072
ROWS_PER_CORE = B_TOTAL // N_CORES  # 16384
FAN = 128
K2 = 2 * FAN  # 256 = 2*fan_in = 2*fan_out
EPS = 1e-6
CHUNKS = [1024] + [2048] * 7 + [1024]
assert sum(CHUNKS) == ROWS_PER_CORE

_NC_CACHE = {}


def _build_nc(rows_per_core):
    from concourse import bacc, mybir, tile

    f32 = mybir.dt.float32
    bf16 = mybir.dt.bfloat16
    i8 = mybir.dt.int8
    Sign = mybir.ActivationFunctionType.Sign
    Copy = mybir.ActivationFunctionType.Copy
    Alu = mybir.AluOpType

    assert rows_per_core == ROWS_PER_CORE
    starts = [sum(CHUNKS[:i]) for i in range(len(CHUNKS))]

    nc = bacc.Bacc("TRN2", target_bir_lowering=False, debug=False)

    # x arrives k-major: row k (0..255), column b = batch row within shard.
    x_d = nc.dram_tensor("x", [K2, rows_per_core], bf16, kind="ExternalInput")
    # weights arrive pre-transposed: wrt[k, o] = weight_real[o, k].
    wrt_d = nc.dram_tensor("weight_real_t", [FAN, FAN], f32, kind="ExternalInput")
    wit_d = nc.dram_tensor("weight_imag_t", [FAN, FAN], f32, kind="ExternalInput")
    # out is produced transposed: out_d[o, b] = out[b, o] / 2.
    out_d = nc.dram_tensor("out", [K2, rows_per_core], i8, kind="ExternalOutput")

    with tile.TileContext(nc) as tc:
        with (
            tc.tile_pool(name="const", bufs=1) as const_pool,
            tc.tile_pool(name="xt", bufs=len(CHUNKS)) as xt_pool,
            tc.tile_pool(name="xb", bufs=3) as xb_pool,
            tc.tile_pool(name="oout", bufs=3) as o_pool,
            tc.tile_pool(name="po", bufs=2, space="PSUM") as po_pool,
        ):
            # Per-chunk input tiles [128, (h, w)]: k-half h on cols h*w + b.
            xt_tiles = []

            def load(c):
                s, w = starts[c], CHUNKS[c]
                xt = xt_pool.tile([128, 2 * w], bf16, tag="xt")
                nc.sync.dma_start(
                    out=xt[:].rearrange("p (h b) -> p h b", h=2),
                    in_=x_d.rearrange("(h p) b -> p h b", h=2)[:, :, s : s + w],
                )
                xt_tiles.append(xt)

            # Get the x stream going before anything else.
            for c in range(len(CHUNKS)):
                load(c)

            # PE warm-up: ~5us of junk matmuls so the HAM clock gate opens
            # (1.2 -> 2.4 GHz) before the first real matmul arrives.
            warm = const_pool.tile([128, 128], bf16)
            nc.vector.memset(warm[:], 1.0)
            warm_ps = po_pool.tile([128, 1024], f32, tag="po", name="warm_ps")
            for _ in range(56):
                nc.tensor.matmul(
                    warm_ps[:, 0:64], warm[:], warm[:, 0:64], start=True, stop=True
                )

            # kernelT [256 k, 256 o] as two [128, 256] bf16 tiles scaled x2
            # (x is binarized to {-.5,+.5}, so products are exactly +-1):
            #   kt0 = 2*[ sign(wr^T) | sign(wi^T) ]   (k in [0,128))
            #   kt1 = 2*[ -sign(wi^T) | sign(wr^T) ]  (k in [128,256))
            # sign(w + eps) = (w >= -eps) * 2 - 1, computed as two DVE
            # tensor_scalar passes (no ACT table load on the critical path).
            w_sb = const_pool.tile([128, 256], f32)
            nc.scalar.dma_start(out=w_sb[:, 0:128], in_=wrt_d[:])
            nc.scalar.dma_start(out=w_sb[:, 128:256], in_=wit_d[:])
            kt0_t = const_pool.tile([128, 256], bf16)
            kt1_t = const_pool.tile([128, 256], bf16)
            # kt0 halves: +sign;  kt1: [-sign(wiT) | +sign(wrT)]
            nc.vector.tensor_scalar(kt0_t[:], w_sb[:], -EPS, 4.0, Alu.is_ge, Alu.mult)
            nc.vector.tensor_scalar(
                kt1_t[:, 0:128], w_sb[:, 128:256], -EPS, -4.0, Alu.is_ge, Alu.mult
            )
            nc.vector.tensor_scalar(
                kt1_t[:, 128:256], w_sb[:, 0:128], -EPS, 4.0, Alu.is_ge, Alu.mult
            )
            kt0 = const_pool.tile([128, 256], bf16)
            kt1 = const_pool.tile([128, 256], bf16)
            nc.vector.tensor_scalar(kt0[:], kt0_t[:], -2.0, None, Alu.add)
            nc.vector.tensor_scalar(kt1[:, 0:128], kt1_t[:, 0:128], 2.0, None, Alu.add)
            nc.vector.tensor_scalar(kt1[:, 128:256], kt1_t[:, 128:256], -2.0, None, Alu.add)
            kts = (kt0, kt1)

            for c in range(len(CHUNKS)):
                s, w = starts[c], CHUNKS[c]
                xbt = xb_pool.tile([128, 2 * w], bf16, tag="xb")
                # One-shot binarize: (x >= 0) - 0.5 -> {-0.5, +0.5} bf16.
                nc.vector.tensor_scalar(
                    xbt[:], xt_tiles[c][:], 0.0, 0.5, Alu.is_ge, Alu.subtract
                )
                ot = o_pool.tile([128, 2 * w], i8, tag="ot")
                otv = ot[:].rearrange("p (s b) -> p s b", s=2)
                for os_half in range(2):
                    # One 4-bank PSUM tile per (chunk, o-half): 8 N=512
                    # matmuls fill it, one FD=w copy drains it.
                    po = po_pool.tile([128, w], f32, tag="po", name=f"po_{c}_{os_half}")
                    for h in range(2):
                        # One stationary (128x128 piece of kernelT) streams
                        # all of this chunk's columns: LDW amortizes.
                        lhsT = kts[h][:, os_half * 128 : (os_half + 1) * 128]
                        for q in range(w // 512):
                            nc.tensor.matmul(
                                po[:, q * 512 : (q + 1) * 512],
                                lhsT,
                                xbt[:, h * w + q * 512 : h * w + (q + 1) * 512],
                                start=(h == 0),
                                stop=(h == 1),
                            )
                    dst = otv[:, os_half, :]
                    # s0 copies of odd chunks on DVE (emitted before the
                    # next chunk's binarize in the FIFO), rest on ACT.
                    if os_half == 0 and c % 2 == 1:
                        nc.vector.tensor_scalar(dst, po[:], 0.5, None, Alu.mult)
                    else:
                        nc.scalar.activation(dst, po[:], Copy, bias=0.0, scale=0.5)
                nc.gpsimd.dma_start(
                    out=out_d.rearrange("(s p) b -> p s b", s=2)[:, :, s : s + w],
                    in_=otv,
                )

    nc.compile()
    return nc


def get_nc(rows_per_core=ROWS_PER_CORE):
    if rows_per_core not in _NC_CACHE:
        _NC_CACHE[rows_per_core] = _build_nc(rows_per_core)
    return _NC_CACHE[rows_per_core]


def kernel(x, weight_real, weight_imag, trace=False, tmpdir=None):
    from concourse import bass_utils

    x = np.asarray(x, dtype=np.float32)
    wr = np.asarray(weight_real, dtype=np.float32)
    wi = np.asarray(weight_imag, dtype=np.float32)
    assert x.shape == (B_TOTAL, K2) and wr.shape == (FAN, FAN) and wi.shape == (FAN, FAN)

    # Fold the +eps into the bf16 cast: sign(bf16(x + eps)) == sign(x + eps)
    # (round-to-nearest never crosses 0; exact-0 results go +1 via the
    # device-side >= 0 test, matching sign(0 + eps)).  Feed each core its
    # shard k-major ([256, 16384]); weights go in pre-transposed.
    x_bf = (x + np.float32(EPS)).astype(ml_dtypes.bfloat16)
    xp = np.ascontiguousarray(
        x_bf.reshape(N_CORES, ROWS_PER_CORE, K2).transpose(0, 2, 1)
    )
    wrt = np.ascontiguousarray(wr.T)
    wit = np.ascontiguousarray(wi.T)

    nc = get_nc()
    in_maps = [
        {"x": xp[i], "weight_real_t": wrt, "weight_imag_t": wit}
        for i in range(N_CORES)
    ]
    res = bass_utils.run_bass_kernel_spmd(
        nc, in_maps, core_ids=list(range(N_CORES)), trace=trace, tmpdir=tmpdir
    )
    # out_d[o, b] = out[b, o]/2 per core: untranspose and upcast.
    out = np.empty((B_TOTAL, K2), dtype=np.float32)
    for i in range(N_CORES):
        np.multiply(
            res.results[i]["out"].T, np.float32(2.0),
            out=out[i * ROWS_PER_CORE : (i + 1) * ROWS_PER_CORE],
        )
    if trace:
        return out, res
    return out
